# revision 25
# baseline (speedup 1.0000x reference)
"""Trainium2 Bass kernel for deformable 3x3 convolution (nn_DeformConvWarp).

Problem: x [4,128,128,128] f32, offset [4,18,128,128] f32 (torchvision layout,
per-tap (dy,dx) interleaved), weight [128,128,3,3] f32.
out[b,o,h,w] = sum_{c,k} W[o,c,k] * bilinear_sample(x[b,c], p_k(h,w)+off_k(h,w))

Sharding: 8 cores = batch (4) x output-row-half (2). Each core computes
out[b, :, h2*64:(h2+1)*64, :] = [128, 8192] f32.

Device algorithm per core:
  - offsets -> (PE transpose to pixel-on-partition layout) -> bilinear
    weights (4 per tap, validity folded in) + 2 gather indices per tap
    (top/bottom row pairs), all in a [128p, 9k, 64blk] layout (p = w-column,
    blk = output row).
  - indices are re-laid out on device into dma_gather's wrapped int16
    format ([j%16, j//16], replicated across the 8 16-partition bands) via
    two PE transpose passes; a host-built "replicated identity" rhs makes
    the band replication free inside the first transpose.
  - dma_gather (SWDGE custom DMA) from the NHWC-padded image xt
    [16386, 128] bf16 in DRAM: chunk = 2 adjacent pixels x 128 channels
    (512B) per (tap, pixel); <=512 idxs per call (descriptor-ring cap),
    calls round-robined over the 4 SWDGE queues (4 Q7 core-pairs
    generate descriptors in parallel).
  - DVE scales the 4 sub-tap tensors by the bilinear weights (bf16 2x mode
    via 8x-expanded weights so the innermost AP dim stays packed).
  - PE transposes (matmul vs identity) the 4 scaled tiles into one PSUM
    accumulator -> patches [c, pix] (the bilinear sum happens in PSUM).
  - PE conv: 9 matmuls (one per tap) accumulate W_k^T @ patches_k,
    fp32 PSUM, N=512 columns per tile.

Measured on 8 axon trn2 cores: rel-l2 error 0.0037 vs the fp32 reference,
HW exec ~588us (NTFF, core 0; down from 1566us for the first correct
version). Remaining bottlenecks, in order: (1) SWDGE descriptor generation
(~8.5ns/idx across 4 parallel Q7 pairs ~= 310us for the 147k idxs/core) --
fundamental to dma_gather; (2) the ~160us serial prologue (offset
transpose -> index/weight arithmetic -> wrap construction, a DVE-bound
dependency chain) before the first gather issues -- would need Tile
priority hints to interleave the per-k-pair wrap slices with early
gathers; (3) PE at ~310us busy (2304 transpose-matmuls + 144 conv
matmuls), already near the issue-rate floor for 128x128 LDW+MM pairs.
"""

import os
import sys
import numpy as np

sys.path.insert(0, "/opt/trn_rl_repo")

import ml_dtypes

bf16 = ml_dtypes.bfloat16

B, C, H, W = 4, 128, 128, 128
O, K = 128, 9
HALF = 64
NPIX = HALF * W          # 8192 pixels per core
NBLK = HALF              # 64 row-blocks of 128 pixels
TBLK = 4                 # row-blocks per pipeline tile
NT = NBLK // TBLK        # 32 tiles
TPIX = TBLK * 128        # 256 pixels per tile
HW = H * W

_CACHE = {}


def _build_nc():
    import concourse.bass as bass
    import concourse.mybir as mybir
    import concourse.tile as tile
    from concourse import bacc
    from concourse.bass import IndirectOffsetOnAxis

    f32 = mybir.dt.float32
    bft = mybir.dt.bfloat16
    i16 = mybir.dt.int16
    Alu = mybir.AluOpType

    nc = bacc.Bacc("TRN2", target_bir_lowering=False, debug=False,
               num_swdge_queues=4)

    xt = nc.declare_dram_parameter("xt", [HW + 2, C], bft, isOutput=False)
    off = nc.declare_dram_parameter("off", [2 * K, NPIX], f32, isOutput=False)
    wt = nc.declare_dram_parameter("wt", [K, C, O], bft, isOutput=False)
    gy = nc.declare_dram_parameter("gy", [128, K, NBLK], f32, isOutput=False)
    gx = nc.declare_dram_parameter("gx", [128, K, NBLK], f32, isOutput=False)
    identb = nc.declare_dram_parameter("identb", [128, 128], bft, isOutput=False)
    identf = nc.declare_dram_parameter("identf", [128, 128], f32, isOutput=False)
    irep = nc.declare_dram_parameter("irep", [128, 1024], f32, isOutput=False)
    out = nc.declare_dram_parameter("out", [O, NPIX], f32, isOutput=True)

    with tile.TileContext(nc) as tc:
        with tc.tile_pool(name="const", bufs=1) as cpool:
            # ---- constants ----
            wt_sb = cpool.tile([C, K, O], bft, tag="wt")
            nc.sync.dma_start(out=wt_sb[:], in_=wt[:].rearrange("k c o -> c k o"))
            ib_sb = cpool.tile([128, 128], bft, tag="identb")
            nc.sync.dma_start(out=ib_sb[:], in_=identb[:])
            if_sb = cpool.tile([128, 128], f32, tag="identf")
            nc.sync.dma_start(out=if_sb[:], in_=identf[:])
            gy_sb = cpool.tile([128, K, NBLK], f32, tag="gy")
            nc.sync.dma_start(out=gy_sb[:], in_=gy[:])
            gx_sb = cpool.tile([128, K, NBLK], f32, tag="gx")
            nc.sync.dma_start(out=gx_sb[:], in_=gx[:])

            # ---- persistent per-pixel tensors ----
            # gather indices, int16 in dma_gather's wrapped layout:
            # idx j at [j%16, j//16], replicated across the 8 16-partition
            # bands (one per Q7 core). Column order per tile (TBLK row-blocks):
            # col = tile*(K*TBLK*8) + k*(TBLK*8) + b*8 + phi, j = col*16 + p16,
            # so gathered chunk j lands on partition phi*16+p16 = pixel%128.
            # per-s (k-pair) wrap tiles: breaks the false whole-tensor
            # dependency so gathers for k in {2s, 2s+1} start as soon as
            # slice s of the wrap is built
            wrap_top = [cpool.tile([128, NT * (64 if s < 4 else 32)], i16,
                                   tag=f"wrapt{s}", name=f"wrapt{s}")
                        for s in range(5)]
            wrap_bot = [cpool.tile([128, NT * (64 if s < 4 else 32)], i16,
                                   tag=f"wrapb{s}", name=f"wrapb{s}")
                        for s in range(5)]
            # 8x-expanded weights in tile-major order:
            # ae[p, t, k*TBLK+b, i] = a[p, k, t*TBLK+b] for i in 0..8 --
            # per-tile scale AP is then [kb, rep, i] (3 free dims) with a
            # packed innermost dim (DVE 2x mode)
            ae00 = cpool.tile([128, NT, K * TBLK, 8], bft, tag="ae00")
            ae01 = cpool.tile([128, NT, K * TBLK, 8], bft, tag="ae01")
            ae10 = cpool.tile([128, NT, K * TBLK, 8], bft, tag="ae10")
            ae11 = cpool.tile([128, NT, K * TBLK, 8], bft, tag="ae11")

            # ---- offset load + transpose to [p, blk, ch] ----
            with (
                tc.tile_pool(name="pre", bufs=1) as pre,
                tc.tile_pool(name="cpsum", bufs=2, space="PSUM") as cpsum,
            ):
                irep_sb = pre.tile([128, 1024], f32, tag="irep", name="irep_sb")
                nc.sync.dma_start(out=irep_sb[:], in_=irep[:])
                off_sb = pre.tile([2 * K, NPIX], f32, tag="off")
                nc.sync.dma_start(out=off_sb[:], in_=off[:])
                off_T = pre.tile([128, NBLK, 2 * K], f32, tag="offT")
                for blk in range(NBLK):
                    pt = cpsum.tile([128, 2 * K], f32, tag="offtp")
                    nc.tensor.matmul(
                        out=pt[:],
                        lhsT=off_sb[:, blk * 128:(blk + 1) * 128],
                        rhs=if_sb[: 2 * K, : 2 * K],
                        start=True, stop=True,
                    )
                    nc.vector.tensor_copy(out=off_T[:, blk, :], in_=pt[:])

                shape = [128, K, NBLK]

                def pt_(tag):
                    return pre.tile(shape, f32, tag=tag, name=tag)

                # strided views of off_T: oy[p,k,blk] = off_T[p, blk, 2k]
                offT_ap = off_T[:]
                oy = bass.AP(offT_ap.tensor, offT_ap.offset,
                             [[offT_ap.ap[0][0], 128], [2, K], [2 * K, NBLK]])
                ox = bass.AP(offT_ap.tensor, offT_ap.offset + 1,
                             [[offT_ap.ap[0][0], 128], [2, K], [2 * K, NBLK]])

                a00 = pre.tile([128, K, NBLK], bft, tag="a00", name="a00")
                a01 = pre.tile([128, K, NBLK], bft, tag="a01", name="a01")
                a10 = pre.tile([128, K, NBLK], bft, tag="a10", name="a10")
                a11 = pre.tile([128, K, NBLK], bft, tag="a11", name="a11")

                MAGIC = 12582912.0  # 1.5 * 2**23: (x+M)-M == round-to-nearest(x)

                # ---- y axis ----
                py = pt_("py")
                nc.vector.tensor_tensor(out=py[:], in0=gy_sb[:], in1=oy, op=Alu.add)
                yr = pt_("yr")
                nc.vector.tensor_scalar(out=yr[:], in0=py[:], scalar1=MAGIC,
                                        scalar2=MAGIC, op0=Alu.add, op1=Alu.subtract)
                ygt = pt_("ygt")
                nc.vector.tensor_tensor(out=ygt[:], in0=yr[:], in1=py[:], op=Alu.is_gt)
                y0f = pt_("y0f")
                nc.vector.tensor_tensor(out=y0f[:], in0=yr[:], in1=ygt[:], op=Alu.subtract)
                wy = pt_("wy")
                nc.vector.tensor_tensor(out=wy[:], in0=py[:], in1=y0f[:], op=Alu.subtract)
                y0c = pt_("y0c")
                nc.vector.tensor_scalar(out=y0c[:], in0=y0f[:], scalar1=0.0,
                                        scalar2=127.0, op0=Alu.max, op1=Alu.min)
                vy0 = pt_("vy0")
                nc.vector.tensor_tensor(out=vy0[:], in0=y0c[:], in1=y0f[:], op=Alu.is_equal)
                ym1 = pt_("ym1")  # clamp(y0f, -1, 126)
                nc.vector.tensor_scalar(out=ym1[:], in0=y0f[:], scalar1=-1.0,
                                        scalar2=126.0, op0=Alu.max, op1=Alu.min)
                vy1 = pt_("vy1")
                nc.vector.tensor_tensor(out=vy1[:], in0=ym1[:], in1=y0f[:], op=Alu.is_equal)
                y1c = pt_("y1c")  # = ym1 + 1 = clamp(y0f+1, 0, 127)
                nc.vector.tensor_scalar(out=y1c[:], in0=ym1[:], scalar1=1.0,
                                        scalar2=None, op0=Alu.add)

                # ---- x axis (pre-biased by +1 via gx) ----
                px = pt_("px")
                nc.vector.tensor_tensor(out=px[:], in0=gx_sb[:], in1=ox, op=Alu.add)
                xr = pt_("xr")
                nc.vector.tensor_scalar(out=xr[:], in0=px[:], scalar1=MAGIC,
                                        scalar2=MAGIC, op0=Alu.add, op1=Alu.subtract)
                xgt = pt_("xgt")
                nc.vector.tensor_tensor(out=xgt[:], in0=xr[:], in1=px[:], op=Alu.is_gt)
                x0f = pt_("x0f")
                nc.vector.tensor_tensor(out=x0f[:], in0=xr[:], in1=xgt[:], op=Alu.subtract)
                wx = pt_("wx")
                nc.vector.tensor_tensor(out=wx[:], in0=px[:], in1=x0f[:], op=Alu.subtract)
                cx0 = pt_("cx0")
                nc.vector.tensor_scalar(out=cx0[:], in0=x0f[:], scalar1=1.0,
                                        scalar2=128.0, op0=Alu.max, op1=Alu.min)
                vx0 = pt_("vx0")
                nc.vector.tensor_tensor(out=vx0[:], in0=cx0[:], in1=x0f[:], op=Alu.is_equal)
                cx1 = pt_("cx1")
                nc.vector.tensor_scalar(out=cx1[:], in0=x0f[:], scalar1=0.0,
                                        scalar2=127.0, op0=Alu.max, op1=Alu.min)
                vx1 = pt_("vx1")
                nc.vector.tensor_tensor(out=vx1[:], in0=cx1[:], in1=x0f[:], op=Alu.is_equal)

                # ---- bilinear weights (validity folded) ----
                omy = pt_("omy")  # 1 - wy
                nc.vector.tensor_scalar(out=omy[:], in0=wy[:], scalar1=-1.0,
                                        scalar2=1.0, op0=Alu.mult, op1=Alu.add)
                omx = pt_("omx")
                nc.vector.tensor_scalar(out=omx[:], in0=wx[:], scalar1=-1.0,
                                        scalar2=1.0, op0=Alu.mult, op1=Alu.add)
                c0 = pt_("c0")
                nc.vector.tensor_tensor(out=c0[:], in0=omy[:], in1=vy0[:], op=Alu.mult)
                c1 = pt_("c1")
                nc.vector.tensor_tensor(out=c1[:], in0=wy[:], in1=vy1[:], op=Alu.mult)
                b0 = pt_("b0")
                nc.vector.tensor_tensor(out=b0[:], in0=omx[:], in1=vx0[:], op=Alu.mult)
                b1 = pt_("b1")
                nc.vector.tensor_tensor(out=b1[:], in0=wx[:], in1=vx1[:], op=Alu.mult)
                nc.vector.tensor_tensor(out=a00[:], in0=c0[:], in1=b0[:], op=Alu.mult)
                nc.vector.tensor_tensor(out=a01[:], in0=c0[:], in1=b1[:], op=Alu.mult)
                nc.vector.tensor_tensor(out=a10[:], in0=c1[:], in1=b0[:], op=Alu.mult)
                nc.vector.tensor_tensor(out=a11[:], in0=c1[:], in1=b1[:], op=Alu.mult)
                for a_s, a_e in ((a00, ae00), (a01, ae01), (a10, ae10), (a11, ae11)):
                    a8 = pre.tile([128, K, NBLK, 8], bft, tag="a8", name="a8")
                    nc.vector.tensor_copy(
                        out=a8[:],
                        in_=a_s[:, :, :, None].to_broadcast([128, K, NBLK, 8]))
                    # reorder [p, k, (t b), i] -> [p, t, (k b), i]
                    a8ap, aeap = a8[:], a_e[:]
                    s_src = bass.AP(a8ap.tensor, a8ap.offset,
                                    [[a8ap.ap[0][0], 128], [NBLK * 8, K],
                                     [TBLK * 8, NT], [1, TBLK * 8]])
                    s_dst = bass.AP(aeap.tensor, aeap.offset,
                                    [[aeap.ap[0][0], 128], [TBLK * 8, K],
                                     [K * TBLK * 8, NT], [1, TBLK * 8]])
                    nc.vector.tensor_copy(out=s_dst, in_=s_src)

                # ---- gather indices (f32, [128 p, K, NBLK]) ----
                it_f = pt_("itf")
                nc.vector.scalar_tensor_tensor(out=it_f[:], in0=y0c[:], scalar=128.0,
                                               in1=x0f[:], op0=Alu.mult, op1=Alu.add)
                nc.vector.tensor_scalar(out=it_f[:], in0=it_f[:], scalar1=0.0,
                                        scalar2=float(HW), op0=Alu.max, op1=Alu.min)
                ib_f = pt_("ibf")
                nc.vector.scalar_tensor_tensor(out=ib_f[:], in0=y1c[:], scalar=128.0,
                                               in1=x0f[:], op0=Alu.mult, op1=Alu.add)
                nc.vector.tensor_scalar(out=ib_f[:], in0=ib_f[:], scalar1=0.0,
                                        scalar2=float(HW), op0=Alu.max, op1=Alu.min)

                # ---- wrap to dma_gather layout via two PE transposes ----
                # src [p=128, kblk=576] -> T1 -> A[kblk, p] -> per-phi T2 ->
                # psum [p16=16, kblk-slice] -> strided DVE copy (f32->i16) to
                # wrap16[p16, col], col = blkhi*144 + k*16 + blklo*8 + phi
                # (blk = blkhi*TBLK + blklo; col order matches per-tile
                # contiguous slices of 144 columns).
                PCOLT = K * TBLK * 8           # 144 cols per tile
                for srcv, wrap128 in ((it_f, wrap_top), (ib_f, wrap_bot)):
                    srcf = srcv.rearrange("p k b -> p (k b)")
                    # T1: transpose idx [p, kblk] with a replicated identity
                    # rhs: psum cols = 4 phi-blocks of (8 bands x 16 p16),
                    # A_rep[kblk, s, half, phiblk*128 + r*16 + p16]
                    #   = idx[(half*4+phiblk)*16 + p16, kblk]
                    A_rep = pre.tile([128, 5, 2, 512], f32, tag="A_rep",
                                     name="A_rep")
                    for s in range(5):
                        n = 128 if s < 4 else 64
                        for half in range(2):
                            psA = cpsum.tile([128, 512], f32, tag="psA",
                                             name="psA")
                            nc.tensor.matmul(
                                out=psA[:n, :],
                                lhsT=srcf[:, 128 * s:128 * s + n],
                                rhs=irep_sb[:, half * 512:(half + 1) * 512],
                                start=True, stop=True)
                            nc.vector.tensor_copy(out=A_rep[:n, s, half, :],
                                                  in_=psA[:n, :])
                    for s in range(5):
                        n = 128 if s < 4 else 64
                        for phi in range(8):
                            ps2 = cpsum.tile([128, 128], f32, tag="ps2", name="ps2")
                            nc.tensor.matmul(
                                out=ps2[:, :n],
                                lhsT=A_rep[:n, s, phi // 4,
                                           (phi % 4) * 128:(phi % 4) * 128 + 128],
                                rhs=if_sb[:n, :n], start=True, stop=True)
                            # psum cols: kblk-rel = k2*64 + blkhi*TBLK + blklo
                            # dst col in wrap_s: t*(nk2*32) + k2*32 + b*8 + phi
                            nk2 = 2 if s < 4 else 1
                            psv = bass.AP(ps2.tensor, ps2.offset,
                                          [[ps2.ap[0][0], 128],
                                           [64, nk2], [TBLK, NBLK // TBLK], [1, TBLK]])
                            wap = wrap128[s][:]
                            dst = bass.AP(wap.tensor, wap.offset + phi,
                                          [[wap.ap[0][0], 128],
                                           [TBLK * 8, nk2],
                                           [nk2 * TBLK * 8, NBLK // TBLK],
                                           [8, TBLK]])
                            nc.vector.tensor_copy(out=dst, in_=psv)

            # ---- main pipeline ----
            with (
                tc.tile_pool(name="gat", bufs=2) as gpool,
                tc.tile_pool(name="sca", bufs=1) as spool,
                tc.tile_pool(name="pat", bufs=2) as ppool,
                tc.tile_pool(name="ost", bufs=3) as opool,
                tc.tile_pool(name="tpsum", bufs=3, space="PSUM") as tpsum,
                tc.tile_pool(name="opsum", bufs=2, space="PSUM") as opsum,
            ):
                # xt viewed as overlapping 2-pixel rows: row i = pixels (i, i+1)
                xtap = xt[:]
                xt_pair = bass.AP(xtap.tensor, 0, [[C, HW + 1], [1, 2 * C]])
                PCOLT_ = K * TBLK * 8        # 288 wrapped cols per tile
                KCHUNK = [(k, k + 1) for k in range(K)]
                _qn = [0]
                for t in range(NT):
                    bsl = slice(t * TBLK, (t + 1) * TBLK)
                    g_top = gpool.tile([128, K, TBLK, 2 * C], bft, tag="g_top")
                    g_bot = gpool.tile([128, K, TBLK, 2 * C], bft, tag="g_bot")
                    for g_tile, wrapt in ((g_top, wrap_top), (g_bot, wrap_bot)):
                        for ks, ke in KCHUNK:
                            k = ks
                            s_, krel = k // 2, k % 2
                            nk2 = 2 if s_ < 4 else 1
                            nidx = 16 * TBLK * 8
                            c0_ = t * (nk2 * 32) + krel * 32
                            nc.gpsimd.dma_gather(
                                out_ap=g_tile[:, ks:ke].rearrange(
                                    "p k b c -> p (k b) c"),
                                in_ap=xt_pair,
                                idxs_ap=wrapt[s_][:, c0_:c0_ + 32],
                                num_idxs=nidx, num_idxs_reg=nidx,
                                elem_size=2 * C, elem_step=C,
                                queue_num=_qn[0] % 4,
                            )
                            _qn[0] += 1

                    s_t0 = spool.tile([128, K, TBLK, C], bft, tag="s_t0")
                    s_t1 = spool.tile([128, K, TBLK, C], bft, tag="s_t1")
                    s_b0 = spool.tile([128, K, TBLK, C], bft, tag="s_b0")
                    s_b1 = spool.tile([128, K, TBLK, C], bft, tag="s_b1")
                    for s_tile, g_tile, lo, a_e in (
                        (s_t0, g_top, 0, ae00), (s_t1, g_top, C, ae01),
                        (s_b0, g_bot, 0, ae10), (s_b1, g_bot, C, ae11),
                    ):
                        aeap = a_e[:]
                        in1 = bass.AP(
                            aeap.tensor, aeap.offset + t * (K * TBLK * 8),
                            [[aeap.ap[0][0], 128], [8, K * TBLK],
                             [0, C // 8], [1, 8]])
                        gap_ = g_tile[:]
                        in0 = bass.AP(
                            gap_.tensor, gap_.offset + lo,
                            [[gap_.ap[0][0], 128], [2 * C, K * TBLK], [1, C]])
                        sap_ = s_tile[:]
                        outap = bass.AP(
                            sap_.tensor, sap_.offset,
                            [[sap_.ap[0][0], 128], [C, K * TBLK], [1, C]])
                        nc.vector.tensor_tensor(
                            out=outap, in0=in0, in1=in1, op=Alu.mult,
                        )

                    patches = ppool.tile([C, TBLK, K, 128], bft, tag="patches")
                    for k in range(K):
                        for bp in range(TBLK // 2):
                            pp = tpsum.tile([128, 2, 128], f32, tag="pp")
                            for b2 in range(2):
                                for i, s_tile in enumerate((s_t0, s_t1, s_b0, s_b1)):
                                    nc.tensor.matmul(
                                        out=pp[:, b2, :],
                                        lhsT=s_tile[:, k, 2 * bp + b2, :],
                                        rhs=ib_sb[:],
                                        start=(i == 0), stop=(i == 3),
                                    )
                            nc.scalar.copy(out=patches[:, 2 * bp:2 * bp + 2, k, :],
                                           in_=pp[:])

                    op_ = opsum.tile([O, TPIX], f32, tag="op")
                    for k in range(K):
                        nc.tensor.matmul(
                            out=op_[:],
                            lhsT=wt_sb[:, k, :],
                            rhs=patches[:, :, k, :],
                            start=(k == 0), stop=(k == K - 1),
                        )
                    o_sb = opool.tile([O, TPIX], f32, tag="o_sb")
                    nc.vector.tensor_copy(out=o_sb[:], in_=op_[:])
                    nc.sync.dma_start(out=out[:, t * TPIX:(t + 1) * TPIX], in_=o_sb[:])

    nc.finalize()
    return nc


def _host_inputs(x, offset, weight):
    """Build the 8 per-core input maps."""
    wT = np.ascontiguousarray(
        weight.reshape(O, C, K).transpose(2, 1, 0)).astype(bf16)  # [k, c, o]
    identb = np.eye(128, dtype=np.float32).astype(bf16)
    identf = np.eye(128, dtype=np.float32)
    # replicated identity: irep[p, half-col (phiblk, r, p16)] with
    # col = phiblk*128 + r*16 + p16 (+ half*512): 1 iff p == (half*4+phiblk)*16+p16
    irep = np.zeros((128, 1024), np.float32)
    for half in range(2):
        for phiblk in range(4):
            for r in range(8):
                for p16 in range(16):
                    p = (half * 4 + phiblk) * 16 + p16
                    irep[p, half * 512 + phiblk * 128 + r * 16 + p16] = 1.0

    kk = np.arange(K)
    ky = (kk // 3 - 1).astype(np.float32)
    kx = (kk % 3 - 1).astype(np.float32)
    blk = np.arange(NBLK, dtype=np.float32)
    p = np.arange(128, dtype=np.float32)
    gys = []
    for h2 in range(2):
        g = h2 * HALF + blk[None, None, :] + ky[None, :, None]
        gys.append(np.ascontiguousarray(
            np.broadcast_to(g, (128, K, NBLK))).astype(np.float32))
    gxc = p[:, None, None] + kx[None, :, None] + 1.0
    gxc = np.ascontiguousarray(
        np.broadcast_to(gxc, (128, K, NBLK))).astype(np.float32)

    in_maps = []
    meta = []
    for b in range(B):
        x_t = x[b].reshape(C, HW).T.astype(bf16)
        x_t_pad = np.zeros((HW + 2, C), bf16)
        x_t_pad[1:-1] = x_t
        for h2 in range(2):
            off_half = np.ascontiguousarray(
                offset[b, :, h2 * HALF:(h2 + 1) * HALF, :].reshape(2 * K, NPIX))
            in_maps.append({
                "xt": x_t_pad, "off": off_half, "wt": wT,
                "gy": gys[h2], "gx": gxc, "identb": identb, "identf": identf,
                "irep": irep,
            })
            meta.append((b, h2))
    return in_maps, meta


def _run(in_maps, trace=False):
    from concourse.bass_utils import run_bass_kernel_spmd

    if "nc" not in _CACHE:
        _CACHE["nc"] = _build_nc()
    nc = _CACHE["nc"]
    return run_bass_kernel_spmd(nc, in_maps, list(range(8)), trace=trace)


def kernel(x, offset, weight):
    x = np.asarray(x, dtype=np.float32)
    offset = np.asarray(offset, dtype=np.float32)
    weight = np.asarray(weight, dtype=np.float32)
    in_maps, meta = _host_inputs(x, offset, weight)
    res = _run(in_maps, trace=bool(int(os.environ.get("DEFORM_TRACE", "0"))))
    _CACHE["last_result"] = res
    out = np.zeros((B, O, H, W), np.float32)
    for i, (b, h2) in enumerate(meta):
        out[b, :, h2 * HALF:(h2 + 1) * HALF, :] = \
            np.asarray(res.results[i]["out"]).reshape(O, HALF, W)
    return out


# revision 26
# speedup vs baseline: 1.0253x; 1.0253x over previous
"""Trainium2 Bass kernel for deformable 3x3 convolution (nn_DeformConvWarp).

Problem: x [4,128,128,128] f32, offset [4,18,128,128] f32 (torchvision layout,
per-tap (dy,dx) interleaved), weight [128,128,3,3] f32.
out[b,o,h,w] = sum_{c,k} W[o,c,k] * bilinear_sample(x[b,c], p_k(h,w)+off_k(h,w))

Sharding: 8 cores = batch (4) x output-row-half (2). Each core computes
out[b, :, h2*64:(h2+1)*64, :] = [128, 8192] f32.

Device algorithm per core:
  - offsets -> (PE transpose to pixel-on-partition layout) -> bilinear
    weights (4 per tap, validity folded in) + 2 gather indices per tap
    (top/bottom row pairs), all in a [128p, 9k, 64blk] layout (p = w-column,
    blk = output row).
  - indices are re-laid out on device into dma_gather's wrapped int16
    format ([j%16, j//16], replicated across the 8 16-partition bands) via
    two PE transpose passes; a host-built "replicated identity" rhs makes
    the band replication free inside the first transpose.
  - dma_gather (SWDGE custom DMA) from the NHWC-padded image xt
    [16386, 128] bf16 in DRAM: chunk = 2 adjacent pixels x 128 channels
    (512B) per (tap, pixel); <=512 idxs per call (descriptor-ring cap),
    calls round-robined over the 4 SWDGE queues (4 Q7 core-pairs
    generate descriptors in parallel).
  - DVE scales the 4 sub-tap tensors by the bilinear weights (bf16 2x mode
    via 8x-expanded weights so the innermost AP dim stays packed).
  - PE transposes (matmul vs identity) the 4 scaled tiles into one PSUM
    accumulator -> patches [c, pix] (the bilinear sum happens in PSUM).
  - PE conv: 9 matmuls (one per tap) accumulate W_k^T @ patches_k,
    fp32 PSUM, N=512 columns per tile.

Measured on 8 axon trn2 cores: rel-l2 error 0.0037 vs the fp32 reference,
HW exec ~588us (NTFF, core 0; down from 1566us for the first correct
version). Remaining bottlenecks, in order: (1) SWDGE descriptor generation
(~8.5ns/idx across 4 parallel Q7 pairs ~= 310us for the 147k idxs/core) --
fundamental to dma_gather; (2) the ~160us serial prologue (offset
transpose -> index/weight arithmetic -> wrap construction, a DVE-bound
dependency chain) before the first gather issues -- would need Tile
priority hints to interleave the per-k-pair wrap slices with early
gathers; (3) PE at ~310us busy (2304 transpose-matmuls + 144 conv
matmuls), already near the issue-rate floor for 128x128 LDW+MM pairs.
"""

import os
import sys
import numpy as np

sys.path.insert(0, "/opt/trn_rl_repo")

import ml_dtypes

bf16 = ml_dtypes.bfloat16

B, C, H, W = 4, 128, 128, 128
O, K = 128, 9
HALF = 64
NPIX = HALF * W          # 8192 pixels per core
NBLK = HALF              # 64 row-blocks of 128 pixels
TBLK = 4                 # row-blocks per pipeline tile
NT = NBLK // TBLK        # 32 tiles
TPIX = TBLK * 128        # 256 pixels per tile
HW = H * W

_CACHE = {}


def _build_nc():
    import concourse.bass as bass
    import concourse.mybir as mybir
    import concourse.tile as tile
    from concourse import bacc
    from concourse.bass import IndirectOffsetOnAxis

    f32 = mybir.dt.float32
    bft = mybir.dt.bfloat16
    i16 = mybir.dt.int16
    Alu = mybir.AluOpType

    nc = bacc.Bacc("TRN2", target_bir_lowering=False, debug=False,
               num_swdge_queues=4)

    xt = nc.declare_dram_parameter("xt", [HW + 2, C], bft, isOutput=False)
    off = nc.declare_dram_parameter("off", [2 * K, NPIX], f32, isOutput=False)
    wt = nc.declare_dram_parameter("wt", [K, C, O], bft, isOutput=False)
    gy = nc.declare_dram_parameter("gy", [128, K, NBLK], f32, isOutput=False)
    gx = nc.declare_dram_parameter("gx", [128, K, NBLK], f32, isOutput=False)
    identb = nc.declare_dram_parameter("identb", [128, 128], bft, isOutput=False)
    identf = nc.declare_dram_parameter("identf", [128, 128], f32, isOutput=False)
    irep = nc.declare_dram_parameter("irep", [128, 1024], f32, isOutput=False)
    out = nc.declare_dram_parameter("out", [O, NPIX], f32, isOutput=True)

    with tile.TileContext(nc) as tc:
        with tc.tile_pool(name="const", bufs=1) as cpool:
            # ---- constants ----
            wt_sb = cpool.tile([C, K, O], bft, tag="wt")
            nc.sync.dma_start(out=wt_sb[:], in_=wt[:].rearrange("k c o -> c k o"))
            ib_sb = cpool.tile([128, 128], bft, tag="identb")
            nc.sync.dma_start(out=ib_sb[:], in_=identb[:])
            if_sb = cpool.tile([128, 128], f32, tag="identf")
            nc.sync.dma_start(out=if_sb[:], in_=identf[:])
            gy_sb = cpool.tile([128, K, NBLK], f32, tag="gy")
            nc.sync.dma_start(out=gy_sb[:], in_=gy[:])
            gx_sb = cpool.tile([128, K, NBLK], f32, tag="gx")
            nc.sync.dma_start(out=gx_sb[:], in_=gx[:])

            # ---- persistent per-pixel tensors ----
            # gather indices, int16 in dma_gather's wrapped layout:
            # idx j at [j%16, j//16], replicated across the 8 16-partition
            # bands (one per Q7 core). Column order per tile (TBLK row-blocks):
            # col = tile*(K*TBLK*8) + k*(TBLK*8) + b*8 + phi, j = col*16 + p16,
            # so gathered chunk j lands on partition phi*16+p16 = pixel%128.
            # per-s (k-pair) wrap tiles: breaks the false whole-tensor
            # dependency so gathers for k in {2s, 2s+1} start as soon as
            # slice s of the wrap is built
            wrap_top = [cpool.tile([128, NT * (64 if s < 4 else 32)], i16,
                                   tag=f"wrapt{s}", name=f"wrapt{s}")
                        for s in range(5)]
            wrap_bot = [cpool.tile([128, NT * (64 if s < 4 else 32)], i16,
                                   tag=f"wrapb{s}", name=f"wrapb{s}")
                        for s in range(5)]
            # 8x-expanded weights in tile-major order:
            # ae[p, t, k*TBLK+b, i] = a[p, k, t*TBLK+b] for i in 0..8 --
            # per-tile scale AP is then [kb, rep, i] (3 free dims) with a
            # packed innermost dim (DVE 2x mode)
            ae00 = cpool.tile([128, NT, K * TBLK, 8], bft, tag="ae00")
            ae01 = cpool.tile([128, NT, K * TBLK, 8], bft, tag="ae01")
            ae10 = cpool.tile([128, NT, K * TBLK, 8], bft, tag="ae10")
            ae11 = cpool.tile([128, NT, K * TBLK, 8], bft, tag="ae11")

            # ---- offset load + transpose to [p, blk, ch] ----
            with (
                tc.tile_pool(name="pre", bufs=1) as pre,
                tc.tile_pool(name="cpsum", bufs=2, space="PSUM") as cpsum,
                tc.tile_pool(name="cpsum2", bufs=4, space="PSUM") as cpsum2,
            ):
                irep_sb = pre.tile([128, 1024], f32, tag="irep", name="irep_sb")
                nc.sync.dma_start(out=irep_sb[:], in_=irep[:])
                off_sb = pre.tile([2 * K, NPIX], f32, tag="off")
                nc.sync.dma_start(out=off_sb[:], in_=off[:])
                off_T = pre.tile([128, NBLK, 2 * K], f32, tag="offT")
                for blk in range(NBLK):
                    pt = cpsum.tile([128, 2 * K], f32, tag="offtp")
                    nc.tensor.matmul(
                        out=pt[:],
                        lhsT=off_sb[:, blk * 128:(blk + 1) * 128],
                        rhs=if_sb[: 2 * K, : 2 * K],
                        start=True, stop=True,
                    )
                    nc.scalar.copy(out=off_T[:, blk, :], in_=pt[:])

                shape = [128, K, NBLK]

                def pt_(tag):
                    return pre.tile(shape, f32, tag=tag, name=tag)

                # strided views of off_T: oy[p,k,blk] = off_T[p, blk, 2k]
                offT_ap = off_T[:]
                oy = bass.AP(offT_ap.tensor, offT_ap.offset,
                             [[offT_ap.ap[0][0], 128], [2, K], [2 * K, NBLK]])
                ox = bass.AP(offT_ap.tensor, offT_ap.offset + 1,
                             [[offT_ap.ap[0][0], 128], [2, K], [2 * K, NBLK]])

                a00 = pre.tile([128, K, NBLK], bft, tag="a00", name="a00")
                a01 = pre.tile([128, K, NBLK], bft, tag="a01", name="a01")
                a10 = pre.tile([128, K, NBLK], bft, tag="a10", name="a10")
                a11 = pre.tile([128, K, NBLK], bft, tag="a11", name="a11")

                MAGIC = 12582912.0  # 1.5 * 2**23: (x+M)-M == round-to-nearest(x)

                # ---- y axis ----
                py = pt_("py")
                nc.vector.tensor_tensor(out=py[:], in0=gy_sb[:], in1=oy, op=Alu.add)
                yr = pt_("yr")
                nc.vector.tensor_scalar(out=yr[:], in0=py[:], scalar1=MAGIC,
                                        scalar2=MAGIC, op0=Alu.add, op1=Alu.subtract)
                ygt = pt_("ygt")
                nc.vector.tensor_tensor(out=ygt[:], in0=yr[:], in1=py[:], op=Alu.is_gt)
                y0f = pt_("y0f")
                nc.vector.tensor_tensor(out=y0f[:], in0=yr[:], in1=ygt[:], op=Alu.subtract)
                wy = pt_("wy")
                nc.vector.tensor_tensor(out=wy[:], in0=py[:], in1=y0f[:], op=Alu.subtract)
                y0c = pt_("y0c")
                nc.vector.tensor_scalar(out=y0c[:], in0=y0f[:], scalar1=0.0,
                                        scalar2=127.0, op0=Alu.max, op1=Alu.min)
                vy0 = pt_("vy0")
                nc.vector.tensor_tensor(out=vy0[:], in0=y0c[:], in1=y0f[:], op=Alu.is_equal)
                ym1 = pt_("ym1")  # clamp(y0f, -1, 126)
                nc.vector.tensor_scalar(out=ym1[:], in0=y0f[:], scalar1=-1.0,
                                        scalar2=126.0, op0=Alu.max, op1=Alu.min)
                vy1 = pt_("vy1")
                nc.vector.tensor_tensor(out=vy1[:], in0=ym1[:], in1=y0f[:], op=Alu.is_equal)
                y1c = pt_("y1c")  # = ym1 + 1 = clamp(y0f+1, 0, 127)
                nc.vector.tensor_scalar(out=y1c[:], in0=ym1[:], scalar1=1.0,
                                        scalar2=None, op0=Alu.add)

                # ---- x axis (pre-biased by +1 via gx) ----
                px = pt_("px")
                nc.vector.tensor_tensor(out=px[:], in0=gx_sb[:], in1=ox, op=Alu.add)
                xr = pt_("xr")
                nc.vector.tensor_scalar(out=xr[:], in0=px[:], scalar1=MAGIC,
                                        scalar2=MAGIC, op0=Alu.add, op1=Alu.subtract)
                xgt = pt_("xgt")
                nc.vector.tensor_tensor(out=xgt[:], in0=xr[:], in1=px[:], op=Alu.is_gt)
                x0f = pt_("x0f")
                nc.vector.tensor_tensor(out=x0f[:], in0=xr[:], in1=xgt[:], op=Alu.subtract)
                wx = pt_("wx")
                nc.vector.tensor_tensor(out=wx[:], in0=px[:], in1=x0f[:], op=Alu.subtract)
                cx0 = pt_("cx0")
                nc.vector.tensor_scalar(out=cx0[:], in0=x0f[:], scalar1=1.0,
                                        scalar2=128.0, op0=Alu.max, op1=Alu.min)
                vx0 = pt_("vx0")
                nc.vector.tensor_tensor(out=vx0[:], in0=cx0[:], in1=x0f[:], op=Alu.is_equal)
                cx1 = pt_("cx1")
                nc.vector.tensor_scalar(out=cx1[:], in0=x0f[:], scalar1=0.0,
                                        scalar2=127.0, op0=Alu.max, op1=Alu.min)
                vx1 = pt_("vx1")
                nc.vector.tensor_tensor(out=vx1[:], in0=cx1[:], in1=x0f[:], op=Alu.is_equal)

                # ---- bilinear weights (validity folded) ----
                omy = pt_("omy")  # 1 - wy
                nc.vector.tensor_scalar(out=omy[:], in0=wy[:], scalar1=-1.0,
                                        scalar2=1.0, op0=Alu.mult, op1=Alu.add)
                omx = pt_("omx")
                nc.vector.tensor_scalar(out=omx[:], in0=wx[:], scalar1=-1.0,
                                        scalar2=1.0, op0=Alu.mult, op1=Alu.add)
                c0 = pt_("c0")
                nc.vector.tensor_tensor(out=c0[:], in0=omy[:], in1=vy0[:], op=Alu.mult)
                c1 = pt_("c1")
                nc.vector.tensor_tensor(out=c1[:], in0=wy[:], in1=vy1[:], op=Alu.mult)
                b0 = pt_("b0")
                nc.vector.tensor_tensor(out=b0[:], in0=omx[:], in1=vx0[:], op=Alu.mult)
                b1 = pt_("b1")
                nc.vector.tensor_tensor(out=b1[:], in0=wx[:], in1=vx1[:], op=Alu.mult)
                nc.vector.tensor_tensor(out=a00[:], in0=c0[:], in1=b0[:], op=Alu.mult)
                nc.vector.tensor_tensor(out=a01[:], in0=c0[:], in1=b1[:], op=Alu.mult)
                nc.vector.tensor_tensor(out=a10[:], in0=c1[:], in1=b0[:], op=Alu.mult)
                nc.vector.tensor_tensor(out=a11[:], in0=c1[:], in1=b1[:], op=Alu.mult)
                for a_s, a_e in ((a00, ae00), (a01, ae01), (a10, ae10), (a11, ae11)):
                    a8 = pre.tile([128, K, NBLK, 8], bft, tag="a8", name="a8")
                    nc.vector.tensor_copy(
                        out=a8[:],
                        in_=a_s[:, :, :, None].to_broadcast([128, K, NBLK, 8]))
                    # reorder [p, k, (t b), i] -> [p, t, (k b), i]
                    a8ap, aeap = a8[:], a_e[:]
                    s_src = bass.AP(a8ap.tensor, a8ap.offset,
                                    [[a8ap.ap[0][0], 128], [NBLK * 8, K],
                                     [TBLK * 8, NT], [1, TBLK * 8]])
                    s_dst = bass.AP(aeap.tensor, aeap.offset,
                                    [[aeap.ap[0][0], 128], [TBLK * 8, K],
                                     [K * TBLK * 8, NT], [1, TBLK * 8]])
                    nc.vector.tensor_copy(out=s_dst, in_=s_src)

                # ---- gather indices (f32, [128 p, K, NBLK]) ----
                it_f = pt_("itf")
                nc.vector.scalar_tensor_tensor(out=it_f[:], in0=y0c[:], scalar=128.0,
                                               in1=x0f[:], op0=Alu.mult, op1=Alu.add)
                nc.vector.tensor_scalar(out=it_f[:], in0=it_f[:], scalar1=0.0,
                                        scalar2=float(HW), op0=Alu.max, op1=Alu.min)
                ib_f = pt_("ibf")
                nc.vector.scalar_tensor_tensor(out=ib_f[:], in0=y1c[:], scalar=128.0,
                                               in1=x0f[:], op0=Alu.mult, op1=Alu.add)
                nc.vector.tensor_scalar(out=ib_f[:], in0=ib_f[:], scalar1=0.0,
                                        scalar2=float(HW), op0=Alu.max, op1=Alu.min)

                # ---- wrap to dma_gather layout via two PE transposes ----
                # src [p=128, kblk=576] -> T1 -> A[kblk, p] -> per-phi T2 ->
                # psum [p16=16, kblk-slice] -> strided DVE copy (f32->i16) to
                # wrap16[p16, col], col = blkhi*144 + k*16 + blklo*8 + phi
                # (blk = blkhi*TBLK + blklo; col order matches per-tile
                # contiguous slices of 144 columns).
                PCOLT = K * TBLK * 8           # 144 cols per tile
                for srcv, wrap128 in ((it_f, wrap_top), (ib_f, wrap_bot)):
                    srcf = srcv.rearrange("p k b -> p (k b)")
                    # T1: transpose idx [p, kblk] with a replicated identity
                    # rhs: psum cols = 4 phi-blocks of (8 bands x 16 p16),
                    # A_rep[kblk, s, half, phiblk*128 + r*16 + p16]
                    #   = idx[(half*4+phiblk)*16 + p16, kblk]
                    A_rep = pre.tile([128, 5, 2, 512], f32, tag="A_rep",
                                     name="A_rep")
                    for s in range(5):
                        n = 128 if s < 4 else 64
                        for half in range(2):
                            psA = cpsum.tile([128, 512], f32, tag="psA",
                                             name="psA")
                            nc.tensor.matmul(
                                out=psA[:n, :],
                                lhsT=srcf[:, 128 * s:128 * s + n],
                                rhs=irep_sb[:, half * 512:(half + 1) * 512],
                                start=True, stop=True)
                            nc.scalar.copy(out=A_rep[:n, s, half, :],
                                           in_=psA[:n, :])
                    for s in range(5):
                        n = 128 if s < 4 else 64
                        for phi in range(8):
                            ps2 = cpsum2.tile([128, 128], f32, tag="ps2", name="ps2")
                            nc.tensor.matmul(
                                out=ps2[:, :n],
                                lhsT=A_rep[:n, s, phi // 4,
                                           (phi % 4) * 128:(phi % 4) * 128 + 128],
                                rhs=if_sb[:n, :n], start=True, stop=True)
                            # psum cols: kblk-rel = k2*64 + blkhi*TBLK + blklo
                            # dst col in wrap_s: t*(nk2*32) + k2*32 + b*8 + phi
                            nk2 = 2 if s < 4 else 1
                            psv = bass.AP(ps2.tensor, ps2.offset,
                                          [[ps2.ap[0][0], 128],
                                           [64, nk2], [TBLK, NBLK // TBLK], [1, TBLK]])
                            wap = wrap128[s][:]
                            dst = bass.AP(wap.tensor, wap.offset + phi,
                                          [[wap.ap[0][0], 128],
                                           [TBLK * 8, nk2],
                                           [nk2 * TBLK * 8, NBLK // TBLK],
                                           [8, TBLK]])
                            nc.vector.tensor_copy(out=dst, in_=psv)

            # ---- main pipeline ----
            with (
                tc.tile_pool(name="gat", bufs=2) as gpool,
                tc.tile_pool(name="sca", bufs=1) as spool,
                tc.tile_pool(name="pat", bufs=2) as ppool,
                tc.tile_pool(name="ost", bufs=3) as opool,
                tc.tile_pool(name="tpsum", bufs=3, space="PSUM") as tpsum,
                tc.tile_pool(name="opsum", bufs=2, space="PSUM") as opsum,
            ):
                # xt viewed as overlapping 2-pixel rows: row i = pixels (i, i+1)
                xtap = xt[:]
                xt_pair = bass.AP(xtap.tensor, 0, [[C, HW + 1], [1, 2 * C]])
                PCOLT_ = K * TBLK * 8        # 288 wrapped cols per tile
                KCHUNK = [(k, k + 1) for k in range(K)]
                _qn = [0]
                for t in range(NT):
                    bsl = slice(t * TBLK, (t + 1) * TBLK)
                    g_top = gpool.tile([128, K, TBLK, 2 * C], bft, tag="g_top")
                    g_bot = gpool.tile([128, K, TBLK, 2 * C], bft, tag="g_bot")
                    for g_tile, wrapt in ((g_top, wrap_top), (g_bot, wrap_bot)):
                        for ks, ke in KCHUNK:
                            k = ks
                            s_, krel = k // 2, k % 2
                            nk2 = 2 if s_ < 4 else 1
                            nidx = 16 * TBLK * 8
                            c0_ = t * (nk2 * 32) + krel * 32
                            nc.gpsimd.dma_gather(
                                out_ap=g_tile[:, ks:ke].rearrange(
                                    "p k b c -> p (k b) c"),
                                in_ap=xt_pair,
                                idxs_ap=wrapt[s_][:, c0_:c0_ + 32],
                                num_idxs=nidx, num_idxs_reg=nidx,
                                elem_size=2 * C, elem_step=C,
                                queue_num=_qn[0] % 4,
                            )
                            _qn[0] += 1

                    s_t0 = spool.tile([128, K, TBLK, C], bft, tag="s_t0")
                    s_t1 = spool.tile([128, K, TBLK, C], bft, tag="s_t1")
                    s_b0 = spool.tile([128, K, TBLK, C], bft, tag="s_b0")
                    s_b1 = spool.tile([128, K, TBLK, C], bft, tag="s_b1")
                    for s_tile, g_tile, lo, a_e in (
                        (s_t0, g_top, 0, ae00), (s_t1, g_top, C, ae01),
                        (s_b0, g_bot, 0, ae10), (s_b1, g_bot, C, ae11),
                    ):
                        aeap = a_e[:]
                        in1 = bass.AP(
                            aeap.tensor, aeap.offset + t * (K * TBLK * 8),
                            [[aeap.ap[0][0], 128], [8, K * TBLK],
                             [0, C // 8], [1, 8]])
                        gap_ = g_tile[:]
                        in0 = bass.AP(
                            gap_.tensor, gap_.offset + lo,
                            [[gap_.ap[0][0], 128], [2 * C, K * TBLK], [1, C]])
                        sap_ = s_tile[:]
                        outap = bass.AP(
                            sap_.tensor, sap_.offset,
                            [[sap_.ap[0][0], 128], [C, K * TBLK], [1, C]])
                        nc.vector.tensor_tensor(
                            out=outap, in0=in0, in1=in1, op=Alu.mult,
                        )

                    patches = ppool.tile([C, TBLK, K, 128], bft, tag="patches")
                    for k in range(K):
                        for bp in range(TBLK // 2):
                            pp = tpsum.tile([128, 2, 128], f32, tag="pp")
                            for b2 in range(2):
                                for i, s_tile in enumerate((s_t0, s_t1, s_b0, s_b1)):
                                    nc.tensor.matmul(
                                        out=pp[:, b2, :],
                                        lhsT=s_tile[:, k, 2 * bp + b2, :],
                                        rhs=ib_sb[:],
                                        start=(i == 0), stop=(i == 3),
                                    )
                            nc.scalar.copy(out=patches[:, 2 * bp:2 * bp + 2, k, :],
                                           in_=pp[:])

                    op_ = opsum.tile([O, TPIX], f32, tag="op")
                    for k in range(K):
                        nc.tensor.matmul(
                            out=op_[:],
                            lhsT=wt_sb[:, k, :],
                            rhs=patches[:, :, k, :],
                            start=(k == 0), stop=(k == K - 1),
                        )
                    o_sb = opool.tile([O, TPIX], f32, tag="o_sb")
                    nc.vector.tensor_copy(out=o_sb[:], in_=op_[:])
                    nc.sync.dma_start(out=out[:, t * TPIX:(t + 1) * TPIX], in_=o_sb[:])

    nc.finalize()
    return nc


def _host_inputs(x, offset, weight):
    """Build the 8 per-core input maps."""
    wT = np.ascontiguousarray(
        weight.reshape(O, C, K).transpose(2, 1, 0)).astype(bf16)  # [k, c, o]
    identb = np.eye(128, dtype=np.float32).astype(bf16)
    identf = np.eye(128, dtype=np.float32)
    # replicated identity: irep[p, half-col (phiblk, r, p16)] with
    # col = phiblk*128 + r*16 + p16 (+ half*512): 1 iff p == (half*4+phiblk)*16+p16
    irep = np.zeros((128, 1024), np.float32)
    for half in range(2):
        for phiblk in range(4):
            for r in range(8):
                for p16 in range(16):
                    p = (half * 4 + phiblk) * 16 + p16
                    irep[p, half * 512 + phiblk * 128 + r * 16 + p16] = 1.0

    kk = np.arange(K)
    ky = (kk // 3 - 1).astype(np.float32)
    kx = (kk % 3 - 1).astype(np.float32)
    blk = np.arange(NBLK, dtype=np.float32)
    p = np.arange(128, dtype=np.float32)
    gys = []
    for h2 in range(2):
        g = h2 * HALF + blk[None, None, :] + ky[None, :, None]
        gys.append(np.ascontiguousarray(
            np.broadcast_to(g, (128, K, NBLK))).astype(np.float32))
    gxc = p[:, None, None] + kx[None, :, None] + 1.0
    gxc = np.ascontiguousarray(
        np.broadcast_to(gxc, (128, K, NBLK))).astype(np.float32)

    in_maps = []
    meta = []
    for b in range(B):
        x_t = x[b].reshape(C, HW).T.astype(bf16)
        x_t_pad = np.zeros((HW + 2, C), bf16)
        x_t_pad[1:-1] = x_t
        for h2 in range(2):
            off_half = np.ascontiguousarray(
                offset[b, :, h2 * HALF:(h2 + 1) * HALF, :].reshape(2 * K, NPIX))
            in_maps.append({
                "xt": x_t_pad, "off": off_half, "wt": wT,
                "gy": gys[h2], "gx": gxc, "identb": identb, "identf": identf,
                "irep": irep,
            })
            meta.append((b, h2))
    return in_maps, meta


def _run(in_maps, trace=False):
    from concourse.bass_utils import run_bass_kernel_spmd

    if "nc" not in _CACHE:
        _CACHE["nc"] = _build_nc()
    nc = _CACHE["nc"]
    return run_bass_kernel_spmd(nc, in_maps, list(range(8)), trace=trace)


def kernel(x, offset, weight):
    x = np.asarray(x, dtype=np.float32)
    offset = np.asarray(offset, dtype=np.float32)
    weight = np.asarray(weight, dtype=np.float32)
    in_maps, meta = _host_inputs(x, offset, weight)
    res = _run(in_maps, trace=bool(int(os.environ.get("DEFORM_TRACE", "0"))))
    _CACHE["last_result"] = res
    out = np.zeros((B, O, H, W), np.float32)
    for i, (b, h2) in enumerate(meta):
        out[b, :, h2 * HALF:(h2 + 1) * HALF, :] = \
            np.asarray(res.results[i]["out"]).reshape(O, HALF, W)
    return out


# revision 28
# speedup vs baseline: 1.0289x; 1.0036x over previous
"""Trainium2 Bass kernel for deformable 3x3 convolution (nn_DeformConvWarp).

Problem: x [4,128,128,128] f32, offset [4,18,128,128] f32 (torchvision layout,
per-tap (dy,dx) interleaved), weight [128,128,3,3] f32.
out[b,o,h,w] = sum_{c,k} W[o,c,k] * bilinear_sample(x[b,c], p_k(h,w)+off_k(h,w))

Sharding: 8 cores = batch (4) x output-row-half (2). Each core computes
out[b, :, h2*64:(h2+1)*64, :] = [128, 8192] f32.

Device algorithm per core:
  - offsets -> (PE transpose to pixel-on-partition layout) -> bilinear
    weights (4 per tap, validity folded in) + 2 gather indices per tap
    (top/bottom row pairs), all in a [128p, 9k, 64blk] layout (p = w-column,
    blk = output row).
  - indices are re-laid out on device into dma_gather's wrapped int16
    format ([j%16, j//16], replicated across the 8 16-partition bands) via
    two PE transpose passes; a host-built "replicated identity" rhs makes
    the band replication free inside the first transpose.
  - dma_gather (SWDGE custom DMA) from the NHWC-padded image xt
    [16386, 128] bf16 in DRAM: chunk = 2 adjacent pixels x 128 channels
    (512B) per (tap, pixel); <=512 idxs per call (descriptor-ring cap),
    calls round-robined over the 4 SWDGE queues (4 Q7 core-pairs
    generate descriptors in parallel).
  - DVE scales the 4 sub-tap tensors by the bilinear weights (bf16 2x mode
    via 8x-expanded weights so the innermost AP dim stays packed).
  - PE transposes (matmul vs identity) the 4 scaled tiles into one PSUM
    accumulator -> patches [c, pix] (the bilinear sum happens in PSUM).
  - PE conv: 9 matmuls (one per tap) accumulate W_k^T @ patches_k,
    fp32 PSUM, N=512 columns per tile.

Measured on 8 axon trn2 cores: rel-l2 error 0.0037 vs the fp32 reference,
HW exec ~580us (NTFF, core 0; down from 1566us for the first correct
version). Remaining bottlenecks, in order: (1) SWDGE descriptor generation
(~8.5ns/idx across 4 parallel Q7 pairs ~= 310us for the 147k idxs/core) --
fundamental to dma_gather; (2) the ~160us serial prologue (offset
transpose -> index/weight arithmetic -> wrap construction, a DVE-bound
dependency chain) before the first gather issues -- would need Tile
priority hints to interleave the per-k-pair wrap slices with early
gathers; (3) PE at ~310us busy (2304 transpose-matmuls + 144 conv
matmuls), already near the issue-rate floor for 128x128 LDW+MM pairs.
"""

import os
import sys
import numpy as np

sys.path.insert(0, "/opt/trn_rl_repo")

import ml_dtypes

bf16 = ml_dtypes.bfloat16

B, C, H, W = 4, 128, 128, 128
O, K = 128, 9
HALF = 64
NPIX = HALF * W          # 8192 pixels per core
NBLK = HALF              # 64 row-blocks of 128 pixels
TBLK = 4                 # row-blocks per pipeline tile
NT = NBLK // TBLK        # 32 tiles
TPIX = TBLK * 128        # 256 pixels per tile
HW = H * W

_CACHE = {}


def _build_nc():
    import concourse.bass as bass
    import concourse.mybir as mybir
    import concourse.tile as tile
    from concourse import bacc
    from concourse.bass import IndirectOffsetOnAxis

    f32 = mybir.dt.float32
    bft = mybir.dt.bfloat16
    i16 = mybir.dt.int16
    Alu = mybir.AluOpType

    nc = bacc.Bacc("TRN2", target_bir_lowering=False, debug=False,
               num_swdge_queues=4)

    xt = nc.declare_dram_parameter("xt", [HW + 2, C], bft, isOutput=False)
    off = nc.declare_dram_parameter("off", [2 * K, NPIX], f32, isOutput=False)
    wt = nc.declare_dram_parameter("wt", [K, C, O], bft, isOutput=False)
    gy = nc.declare_dram_parameter("gy", [128, K, NBLK], f32, isOutput=False)
    gx = nc.declare_dram_parameter("gx", [128, K, NBLK], f32, isOutput=False)
    identb = nc.declare_dram_parameter("identb", [128, 128], bft, isOutput=False)
    identf = nc.declare_dram_parameter("identf", [128, 128], f32, isOutput=False)
    irep = nc.declare_dram_parameter("irep", [128, 1024], f32, isOutput=False)
    out = nc.declare_dram_parameter("out", [O, NPIX], f32, isOutput=True)

    with tile.TileContext(nc) as tc:
        with tc.tile_pool(name="const", bufs=1) as cpool:
            # ---- constants ----
            wt_sb = cpool.tile([C, K, O], bft, tag="wt")
            nc.sync.dma_start(out=wt_sb[:], in_=wt[:].rearrange("k c o -> c k o"))
            ib_sb = cpool.tile([128, 128], bft, tag="identb")
            nc.sync.dma_start(out=ib_sb[:], in_=identb[:])
            if_sb = cpool.tile([128, 128], f32, tag="identf")
            nc.sync.dma_start(out=if_sb[:], in_=identf[:])
            gy_sb = cpool.tile([128, K, NBLK], f32, tag="gy")
            nc.sync.dma_start(out=gy_sb[:], in_=gy[:])
            gx_sb = cpool.tile([128, K, NBLK], f32, tag="gx")
            nc.sync.dma_start(out=gx_sb[:], in_=gx[:])

            # ---- persistent per-pixel tensors ----
            # gather indices, int16 in dma_gather's wrapped layout:
            # idx j at [j%16, j//16], replicated across the 8 16-partition
            # bands (one per Q7 core). Column order per tile (TBLK row-blocks):
            # col = tile*(K*TBLK*8) + k*(TBLK*8) + b*8 + phi, j = col*16 + p16,
            # so gathered chunk j lands on partition phi*16+p16 = pixel%128.
            # per-s (k-pair) wrap tiles: breaks the false whole-tensor
            # dependency so gathers for k in {2s, 2s+1} start as soon as
            # slice s of the wrap is built
            wrap_top = [cpool.tile([128, NT * (64 if s < 4 else 32)], i16,
                                   tag=f"wrapt{s}", name=f"wrapt{s}")
                        for s in range(5)]
            wrap_bot = [cpool.tile([128, NT * (64 if s < 4 else 32)], i16,
                                   tag=f"wrapb{s}", name=f"wrapb{s}")
                        for s in range(5)]
            # 8x-expanded weights in tile-major order:
            # ae[p, t, k*TBLK+b, i] = a[p, k, t*TBLK+b] for i in 0..8 --
            # per-tile scale AP is then [kb, rep, i] (3 free dims) with a
            # packed innermost dim (DVE 2x mode)
            ae00 = cpool.tile([128, NT, K * TBLK, 8], bft, tag="ae00")
            ae01 = cpool.tile([128, NT, K * TBLK, 8], bft, tag="ae01")
            ae10 = cpool.tile([128, NT, K * TBLK, 8], bft, tag="ae10")
            ae11 = cpool.tile([128, NT, K * TBLK, 8], bft, tag="ae11")

            # ---- offset load + transpose to [p, blk, ch] ----
            with (
                tc.tile_pool(name="pre", bufs=1) as pre,
                tc.tile_pool(name="cpsum", bufs=2, space="PSUM") as cpsum,
                tc.tile_pool(name="cpsum2", bufs=4, space="PSUM") as cpsum2,
            ):
                irep_sb = pre.tile([128, 1024], f32, tag="irep", name="irep_sb")
                nc.sync.dma_start(out=irep_sb[:], in_=irep[:])
                off_sb = pre.tile([2 * K, NPIX], f32, tag="off")
                nc.sync.dma_start(out=off_sb[:], in_=off[:])
                off_T = pre.tile([128, NBLK, 2 * K], f32, tag="offT")
                for blk in range(NBLK):
                    pt = cpsum.tile([128, 2 * K], f32, tag="offtp")
                    nc.tensor.matmul(
                        out=pt[:],
                        lhsT=off_sb[:, blk * 128:(blk + 1) * 128],
                        rhs=if_sb[: 2 * K, : 2 * K],
                        start=True, stop=True,
                    )
                    nc.scalar.copy(out=off_T[:, blk, :], in_=pt[:])

                shape = [128, K, NBLK]

                def pt_(tag):
                    return pre.tile(shape, f32, tag=tag, name=tag)

                # strided views of off_T: oy[p,k,blk] = off_T[p, blk, 2k]
                offT_ap = off_T[:]
                oy = bass.AP(offT_ap.tensor, offT_ap.offset,
                             [[offT_ap.ap[0][0], 128], [2, K], [2 * K, NBLK]])
                ox = bass.AP(offT_ap.tensor, offT_ap.offset + 1,
                             [[offT_ap.ap[0][0], 128], [2, K], [2 * K, NBLK]])

                a00 = pre.tile([128, K, NBLK], bft, tag="a00", name="a00")
                a01 = pre.tile([128, K, NBLK], bft, tag="a01", name="a01")
                a10 = pre.tile([128, K, NBLK], bft, tag="a10", name="a10")
                a11 = pre.tile([128, K, NBLK], bft, tag="a11", name="a11")

                MAGIC = 12582912.0  # 1.5 * 2**23: (x+M)-M == round-to-nearest(x)

                # ---- y axis ----
                py = pt_("py")
                nc.vector.tensor_tensor(out=py[:], in0=gy_sb[:], in1=oy, op=Alu.add)
                yr = pt_("yr")
                nc.vector.tensor_scalar(out=yr[:], in0=py[:], scalar1=MAGIC,
                                        scalar2=MAGIC, op0=Alu.add, op1=Alu.subtract)
                ygt = pt_("ygt")
                nc.vector.tensor_tensor(out=ygt[:], in0=yr[:], in1=py[:], op=Alu.is_gt)
                y0f = pt_("y0f")
                nc.vector.tensor_tensor(out=y0f[:], in0=yr[:], in1=ygt[:], op=Alu.subtract)
                wy = pt_("wy")
                nc.vector.tensor_tensor(out=wy[:], in0=py[:], in1=y0f[:], op=Alu.subtract)
                y0c = pt_("y0c")
                nc.vector.tensor_scalar(out=y0c[:], in0=y0f[:], scalar1=0.0,
                                        scalar2=127.0, op0=Alu.max, op1=Alu.min)
                vy0 = pt_("vy0")
                nc.vector.tensor_tensor(out=vy0[:], in0=y0c[:], in1=y0f[:], op=Alu.is_equal)
                ym1 = pt_("ym1")  # clamp(y0f, -1, 126)
                nc.vector.tensor_scalar(out=ym1[:], in0=y0f[:], scalar1=-1.0,
                                        scalar2=126.0, op0=Alu.max, op1=Alu.min)
                vy1 = pt_("vy1")
                nc.vector.tensor_tensor(out=vy1[:], in0=ym1[:], in1=y0f[:], op=Alu.is_equal)
                y1c = pt_("y1c")  # = ym1 + 1 = clamp(y0f+1, 0, 127)
                nc.vector.tensor_scalar(out=y1c[:], in0=ym1[:], scalar1=1.0,
                                        scalar2=None, op0=Alu.add)

                # ---- x axis (pre-biased by +1 via gx) ----
                px = pt_("px")
                nc.vector.tensor_tensor(out=px[:], in0=gx_sb[:], in1=ox, op=Alu.add)
                xr = pt_("xr")
                nc.vector.tensor_scalar(out=xr[:], in0=px[:], scalar1=MAGIC,
                                        scalar2=MAGIC, op0=Alu.add, op1=Alu.subtract)
                xgt = pt_("xgt")
                nc.vector.tensor_tensor(out=xgt[:], in0=xr[:], in1=px[:], op=Alu.is_gt)
                x0f = pt_("x0f")
                nc.vector.tensor_tensor(out=x0f[:], in0=xr[:], in1=xgt[:], op=Alu.subtract)
                wx = pt_("wx")
                nc.vector.tensor_tensor(out=wx[:], in0=px[:], in1=x0f[:], op=Alu.subtract)
                cx0 = pt_("cx0")
                nc.vector.tensor_scalar(out=cx0[:], in0=x0f[:], scalar1=1.0,
                                        scalar2=128.0, op0=Alu.max, op1=Alu.min)
                vx0 = pt_("vx0")
                nc.vector.tensor_tensor(out=vx0[:], in0=cx0[:], in1=x0f[:], op=Alu.is_equal)
                cx1 = pt_("cx1")
                nc.vector.tensor_scalar(out=cx1[:], in0=x0f[:], scalar1=0.0,
                                        scalar2=127.0, op0=Alu.max, op1=Alu.min)
                vx1 = pt_("vx1")
                nc.vector.tensor_tensor(out=vx1[:], in0=cx1[:], in1=x0f[:], op=Alu.is_equal)

                # ---- bilinear weights (validity folded) ----
                omy = pt_("omy")  # 1 - wy
                nc.vector.tensor_scalar(out=omy[:], in0=wy[:], scalar1=-1.0,
                                        scalar2=1.0, op0=Alu.mult, op1=Alu.add)
                omx = pt_("omx")
                nc.vector.tensor_scalar(out=omx[:], in0=wx[:], scalar1=-1.0,
                                        scalar2=1.0, op0=Alu.mult, op1=Alu.add)
                c0 = pt_("c0")
                nc.vector.tensor_tensor(out=c0[:], in0=omy[:], in1=vy0[:], op=Alu.mult)
                c1 = pt_("c1")
                nc.vector.tensor_tensor(out=c1[:], in0=wy[:], in1=vy1[:], op=Alu.mult)
                b0 = pt_("b0")
                nc.vector.tensor_tensor(out=b0[:], in0=omx[:], in1=vx0[:], op=Alu.mult)
                b1 = pt_("b1")
                nc.vector.tensor_tensor(out=b1[:], in0=wx[:], in1=vx1[:], op=Alu.mult)
                nc.vector.tensor_tensor(out=a00[:], in0=c0[:], in1=b0[:], op=Alu.mult)
                nc.vector.tensor_tensor(out=a01[:], in0=c0[:], in1=b1[:], op=Alu.mult)
                nc.vector.tensor_tensor(out=a10[:], in0=c1[:], in1=b0[:], op=Alu.mult)
                nc.vector.tensor_tensor(out=a11[:], in0=c1[:], in1=b1[:], op=Alu.mult)
                for a_s, a_e in ((a00, ae00), (a01, ae01), (a10, ae10), (a11, ae11)):
                    a8 = pre.tile([128, K, NBLK, 8], bft, tag="a8", name="a8")
                    nc.vector.tensor_copy(
                        out=a8[:],
                        in_=a_s[:, :, :, None].to_broadcast([128, K, NBLK, 8]))
                    # reorder [p, k, (t b), i] -> [p, t, (k b), i]
                    a8ap, aeap = a8[:], a_e[:]
                    s_src = bass.AP(a8ap.tensor, a8ap.offset,
                                    [[a8ap.ap[0][0], 128], [NBLK * 8, K],
                                     [TBLK * 8, NT], [1, TBLK * 8]])
                    s_dst = bass.AP(aeap.tensor, aeap.offset,
                                    [[aeap.ap[0][0], 128], [TBLK * 8, K],
                                     [K * TBLK * 8, NT], [1, TBLK * 8]])
                    nc.vector.tensor_copy(out=s_dst, in_=s_src)

                # ---- gather indices (f32, [128 p, K, NBLK]) ----
                it_f = pt_("itf")
                nc.vector.scalar_tensor_tensor(out=it_f[:], in0=y0c[:], scalar=128.0,
                                               in1=x0f[:], op0=Alu.mult, op1=Alu.add)
                nc.vector.tensor_scalar(out=it_f[:], in0=it_f[:], scalar1=0.0,
                                        scalar2=float(HW), op0=Alu.max, op1=Alu.min)
                ib_f = pt_("ibf")
                nc.vector.scalar_tensor_tensor(out=ib_f[:], in0=y1c[:], scalar=128.0,
                                               in1=x0f[:], op0=Alu.mult, op1=Alu.add)
                nc.vector.tensor_scalar(out=ib_f[:], in0=ib_f[:], scalar1=0.0,
                                        scalar2=float(HW), op0=Alu.max, op1=Alu.min)

                # ---- wrap to dma_gather layout via two PE transposes ----
                # src [p=128, kblk=576] -> T1 -> A[kblk, p] -> per-phi T2 ->
                # psum [p16=16, kblk-slice] -> strided DVE copy (f32->i16) to
                # wrap16[p16, col], col = blkhi*144 + k*16 + blklo*8 + phi
                # (blk = blkhi*TBLK + blklo; col order matches per-tile
                # contiguous slices of 144 columns).
                PCOLT = K * TBLK * 8           # 144 cols per tile
                for srcv, wrap128 in ((it_f, wrap_top), (ib_f, wrap_bot)):
                    srcf = srcv.rearrange("p k b -> p (k b)")
                    # T1: transpose idx [p, kblk] with a replicated identity
                    # rhs: psum cols = 4 phi-blocks of (8 bands x 16 p16),
                    # A_rep[kblk, s, half, phiblk*128 + r*16 + p16]
                    #   = idx[(half*4+phiblk)*16 + p16, kblk]
                    A_rep = pre.tile([128, 5, 2, 512], f32, tag="A_rep",
                                     name="A_rep")
                    for s in range(5):
                        n = 128 if s < 4 else 64
                        for half in range(2):
                            psA = cpsum.tile([128, 512], f32, tag="psA",
                                             name="psA")
                            nc.tensor.matmul(
                                out=psA[:n, :],
                                lhsT=srcf[:, 128 * s:128 * s + n],
                                rhs=irep_sb[:, half * 512:(half + 1) * 512],
                                start=True, stop=True)
                            nc.scalar.copy(out=A_rep[:n, s, half, :],
                                           in_=psA[:n, :])
                    for s in range(5):
                        n = 128 if s < 4 else 64
                        for phi in range(8):
                            ps2 = cpsum2.tile([128, 128], f32, tag="ps2", name="ps2")
                            nc.tensor.matmul(
                                out=ps2[:, :n],
                                lhsT=A_rep[:n, s, phi // 4,
                                           (phi % 4) * 128:(phi % 4) * 128 + 128],
                                rhs=if_sb[:n, :n], start=True, stop=True)
                            # psum cols: kblk-rel = k2*64 + blkhi*TBLK + blklo
                            # dst col in wrap_s: t*(nk2*32) + k2*32 + b*8 + phi
                            nk2 = 2 if s < 4 else 1
                            psv = bass.AP(ps2.tensor, ps2.offset,
                                          [[ps2.ap[0][0], 128],
                                           [64, nk2], [TBLK, NBLK // TBLK], [1, TBLK]])
                            wap = wrap128[s][:]
                            dst = bass.AP(wap.tensor, wap.offset + phi,
                                          [[wap.ap[0][0], 128],
                                           [TBLK * 8, nk2],
                                           [nk2 * TBLK * 8, NBLK // TBLK],
                                           [8, TBLK]])
                            nc.vector.tensor_copy(out=dst, in_=psv)

            # ---- main pipeline ----
            with (
                tc.tile_pool(name="gat", bufs=2) as gpool,
                tc.tile_pool(name="sca", bufs=1) as spool,
                tc.tile_pool(name="pat", bufs=2) as ppool,
                tc.tile_pool(name="ost", bufs=3) as opool,
                tc.tile_pool(name="tpsum", bufs=3, space="PSUM") as tpsum,
                tc.tile_pool(name="opsum", bufs=2, space="PSUM") as opsum,
            ):
                # xt viewed as overlapping 2-pixel rows: row i = pixels (i, i+1)
                xtap = xt[:]
                xt_pair = bass.AP(xtap.tensor, 0, [[C, HW + 1], [1, 2 * C]])
                PCOLT_ = K * TBLK * 8        # 288 wrapped cols per tile
                KCHUNK = [(k, k + 1) for k in range(K)]
                _qn = [0]
                hp = tc.high_priority()
                hp.__enter__()
                for t in range(NT):
                    bsl = slice(t * TBLK, (t + 1) * TBLK)
                    g_top = gpool.tile([128, K, TBLK, 2 * C], bft, tag="g_top")
                    g_bot = gpool.tile([128, K, TBLK, 2 * C], bft, tag="g_bot")
                    for g_tile, wrapt in ((g_top, wrap_top), (g_bot, wrap_bot)):
                        for ks, ke in KCHUNK:
                            k = ks
                            s_, krel = k // 2, k % 2
                            nk2 = 2 if s_ < 4 else 1
                            nidx = 16 * TBLK * 8
                            c0_ = t * (nk2 * 32) + krel * 32
                            nc.gpsimd.dma_gather(
                                out_ap=g_tile[:, ks:ke].rearrange(
                                    "p k b c -> p (k b) c"),
                                in_ap=xt_pair,
                                idxs_ap=wrapt[s_][:, c0_:c0_ + 32],
                                num_idxs=nidx, num_idxs_reg=nidx,
                                elem_size=2 * C, elem_step=C,
                                queue_num=_qn[0] % 4,
                            )
                            _qn[0] += 1

                    s_t0 = spool.tile([128, K, TBLK, C], bft, tag="s_t0")
                    s_t1 = spool.tile([128, K, TBLK, C], bft, tag="s_t1")
                    s_b0 = spool.tile([128, K, TBLK, C], bft, tag="s_b0")
                    s_b1 = spool.tile([128, K, TBLK, C], bft, tag="s_b1")
                    for s_tile, g_tile, lo, a_e in (
                        (s_t0, g_top, 0, ae00), (s_t1, g_top, C, ae01),
                        (s_b0, g_bot, 0, ae10), (s_b1, g_bot, C, ae11),
                    ):
                        aeap = a_e[:]
                        in1 = bass.AP(
                            aeap.tensor, aeap.offset + t * (K * TBLK * 8),
                            [[aeap.ap[0][0], 128], [8, K * TBLK],
                             [0, C // 8], [1, 8]])
                        gap_ = g_tile[:]
                        in0 = bass.AP(
                            gap_.tensor, gap_.offset + lo,
                            [[gap_.ap[0][0], 128], [2 * C, K * TBLK], [1, C]])
                        sap_ = s_tile[:]
                        outap = bass.AP(
                            sap_.tensor, sap_.offset,
                            [[sap_.ap[0][0], 128], [C, K * TBLK], [1, C]])
                        nc.vector.tensor_tensor(
                            out=outap, in0=in0, in1=in1, op=Alu.mult,
                        )

                    patches = ppool.tile([C, TBLK, K, 128], bft, tag="patches")
                    for k in range(K):
                        for bp in range(TBLK // 2):
                            pp = tpsum.tile([128, 2, 128], f32, tag="pp")
                            for b2 in range(2):
                                for i, s_tile in enumerate((s_t0, s_t1, s_b0, s_b1)):
                                    nc.tensor.matmul(
                                        out=pp[:, b2, :],
                                        lhsT=s_tile[:, k, 2 * bp + b2, :],
                                        rhs=ib_sb[:],
                                        start=(i == 0), stop=(i == 3),
                                    )
                            nc.scalar.copy(out=patches[:, 2 * bp:2 * bp + 2, k, :],
                                           in_=pp[:])

                    op_ = opsum.tile([O, TPIX], f32, tag="op")
                    for k in range(K):
                        nc.tensor.matmul(
                            out=op_[:],
                            lhsT=wt_sb[:, k, :],
                            rhs=patches[:, :, k, :],
                            start=(k == 0), stop=(k == K - 1),
                        )
                    o_sb = opool.tile([O, TPIX], f32, tag="o_sb")
                    nc.vector.tensor_copy(out=o_sb[:], in_=op_[:])
                    nc.sync.dma_start(out=out[:, t * TPIX:(t + 1) * TPIX], in_=o_sb[:])
                hp.__exit__(None, None, None)

    nc.finalize()
    return nc


def _host_inputs(x, offset, weight):
    """Build the 8 per-core input maps."""
    wT = np.ascontiguousarray(
        weight.reshape(O, C, K).transpose(2, 1, 0)).astype(bf16)  # [k, c, o]
    identb = np.eye(128, dtype=np.float32).astype(bf16)
    identf = np.eye(128, dtype=np.float32)
    # replicated identity: irep[p, half-col (phiblk, r, p16)] with
    # col = phiblk*128 + r*16 + p16 (+ half*512): 1 iff p == (half*4+phiblk)*16+p16
    irep = np.zeros((128, 1024), np.float32)
    for half in range(2):
        for phiblk in range(4):
            for r in range(8):
                for p16 in range(16):
                    p = (half * 4 + phiblk) * 16 + p16
                    irep[p, half * 512 + phiblk * 128 + r * 16 + p16] = 1.0

    kk = np.arange(K)
    ky = (kk // 3 - 1).astype(np.float32)
    kx = (kk % 3 - 1).astype(np.float32)
    blk = np.arange(NBLK, dtype=np.float32)
    p = np.arange(128, dtype=np.float32)
    gys = []
    for h2 in range(2):
        g = h2 * HALF + blk[None, None, :] + ky[None, :, None]
        gys.append(np.ascontiguousarray(
            np.broadcast_to(g, (128, K, NBLK))).astype(np.float32))
    gxc = p[:, None, None] + kx[None, :, None] + 1.0
    gxc = np.ascontiguousarray(
        np.broadcast_to(gxc, (128, K, NBLK))).astype(np.float32)

    in_maps = []
    meta = []
    for b in range(B):
        x_t = x[b].reshape(C, HW).T.astype(bf16)
        x_t_pad = np.zeros((HW + 2, C), bf16)
        x_t_pad[1:-1] = x_t
        for h2 in range(2):
            off_half = np.ascontiguousarray(
                offset[b, :, h2 * HALF:(h2 + 1) * HALF, :].reshape(2 * K, NPIX))
            in_maps.append({
                "xt": x_t_pad, "off": off_half, "wt": wT,
                "gy": gys[h2], "gx": gxc, "identb": identb, "identf": identf,
                "irep": irep,
            })
            meta.append((b, h2))
    return in_maps, meta


def _run(in_maps, trace=False):
    from concourse.bass_utils import run_bass_kernel_spmd

    if "nc" not in _CACHE:
        _CACHE["nc"] = _build_nc()
    nc = _CACHE["nc"]
    return run_bass_kernel_spmd(nc, in_maps, list(range(8)), trace=trace)


def kernel(x, offset, weight):
    x = np.asarray(x, dtype=np.float32)
    offset = np.asarray(offset, dtype=np.float32)
    weight = np.asarray(weight, dtype=np.float32)
    in_maps, meta = _host_inputs(x, offset, weight)
    res = _run(in_maps, trace=bool(int(os.environ.get("DEFORM_TRACE", "0"))))
    _CACHE["last_result"] = res
    out = np.zeros((B, O, H, W), np.float32)
    for i, (b, h2) in enumerate(meta):
        out[b, :, h2 * HALF:(h2 + 1) * HALF, :] = \
            np.asarray(res.results[i]["out"]).reshape(O, HALF, W)
    return out
